# revision 3
# baseline (speedup 1.0000x reference)
"""Trainium2 Bass kernel for nn_ChebychevInput.

out[b,o,s] = sum_{i,p} (WEIGHT_MAGNITUDE*coef[o,i,p]) * cos(p*arccos(x[b,i,s]))

Device pipeline per core (s-shard of 16384, both batches):
  theta-stage (tiny, flat [96,1024] layout):
      a = arctan(x/sqrt(1-x^2)) = arcsin(x);  theta = pi/2 - a
      theta' = theta * 2^16/(2pi)   (cycles in 2^16 units)
  per (b, s-chunk):
      GPSIMD partition_broadcast -> th3[126, SC] (42 rows per i)
      DVE   (x7 k-tiles): Y32 = int32(th3 * p + 0.25*2^16)   [one pass]
      ACT   one Sin over the int16-bitcast low halfwords: T = sin(2pi*Y/2^16)
            = cos(2pi * p*theta/(2pi)) = cos(p*theta)   -> fp16
      PE    out[o,s] accumulated over 7 k-tiles: lhsT = W[126,128] fp16
      DVE   PSUM -> SBUF fp32, DMA -> out
Row packing: k-tile kt row j: i = j//42, p = 42*kt + j%42  (k=126 rows/tile).
"""
import sys

sys.path.insert(0, "/opt/trn_rl_repo")

import numpy as np

BATCH = 2
INPUT_DIM = 3
N_SAMPLES = 131072
OUTPUT_DIM = 256
POLY_DEGREE = 256  # p = 0..256 -> 257 values
N_CORES = 8
S_SHARD = N_SAMPLES // N_CORES  # 16384
SC = 1024                       # sample chunk
NSC = S_SHARD // SC             # 16
NKT = 7                         # k-tiles of 126 rows (3i x 42p)
KT_ROWS = 126
WEIGHT_MAGNITUDE = float(np.sqrt(6.0 / (INPUT_DIM * (POLY_DEGREE + 1))))
TWO16 = 65536.0

_compiled = {}


def _build():
    import concourse.tile as tile
    from concourse import bacc, mybir

    F32 = mybir.dt.float32
    F16 = mybir.dt.float16
    I32 = mybir.dt.int32
    I16 = mybir.dt.int16
    AF = mybir.ActivationFunctionType
    ALU = mybir.AluOpType

    nc = bacc.Bacc("TRN2", target_bir_lowering=False, debug=False)
    x_d = nc.dram_tensor("x", [BATCH, INPUT_DIM, S_SHARD], F32, kind="ExternalInput")
    w_d = nc.dram_tensor("w", [KT_ROWS, NKT * OUTPUT_DIM], F16, kind="ExternalInput")
    pc_d = nc.dram_tensor("pc", [KT_ROWS, NKT], F32, kind="ExternalInput")
    out_d = nc.dram_tensor("out", [BATCH, OUTPUT_DIM, S_SHARD], F32, kind="ExternalOutput")

    with tile.TileContext(nc) as tc:
        with (
            tc.tile_pool(name="const", bufs=1) as constp,
            tc.tile_pool(name="theta", bufs=1) as thp,
            tc.tile_pool(name="bcast", bufs=2) as bcp,
            tc.tile_pool(name="yint", bufs=2) as yp,
            tc.tile_pool(name="tmat", bufs=2) as tp,
            tc.tile_pool(name="outs", bufs=4) as op,
            tc.tile_pool(name="psum", bufs=4, space="PSUM") as pp,
        ):
            w_t = constp.tile([KT_ROWS, NKT * OUTPUT_DIM], F16)
            nc.sync.dma_start(w_t[:], w_d[:])
            pc_t = constp.tile([KT_ROWS, NKT], F32)
            nc.sync.dma_start(pc_t[:], pc_d[:])

            # ---- theta stage: flat [96, 1024]; row = 48*b + 16*i + u, u = s-chunk
            xt = thp.tile([96, 1024], F32)
            nc.sync.dma_start(xt[:], x_d[:].rearrange("b i (u c) -> (b i u) c", c=1024))
            sq = thp.tile([96, 1024], F32)
            nc.scalar.activation(sq[:], xt[:], AF.Square)
            r2 = thp.tile([96, 1024], F32)
            nc.scalar.activation(r2[:], sq[:], AF.Sqrt, bias=1.0, scale=-1.0)
            inv = thp.tile([96, 1024], F32)
            nc.vector.reciprocal(inv[:], r2[:])
            q = thp.tile([96, 1024], F32)
            nc.vector.tensor_mul(q[:], xt[:], inv[:])
            asn = thp.tile([96, 1024], F32)
            nc.scalar.activation(asn[:], q[:], AF.Arctan)
            # theta' = (pi/2 - a) * 2^16/(2pi) = 2^14 - a * (2^16/2pi)
            thf = thp.tile([96, 1024], F32)
            nc.scalar.activation(thf[:], asn[:], AF.Copy,
                                 bias=16384.0, scale=float(-TWO16 / (2 * np.pi)))

            # ---- main loops
            for b in range(BATCH):
                for sc in range(NSC):
                    th3 = bcp.tile([KT_ROWS, SC], F32)
                    for i in range(INPUT_DIM):
                        row = 48 * b + 16 * i + sc
                        tmp = bcp.tile([1, SC], F32, tag=f"throw{i}")
                        nc.sync.dma_start(tmp[:], thf[row:row + 1, :])
                        bc = bcp.tile([42, SC], F32, tag=f"thbc{i}")
                        nc.gpsimd.partition_broadcast(bc[:], tmp[:])
                        nc.sync.dma_start(th3[42 * i:42 * (i + 1), :], bc[:])
                    y32 = yp.tile([KT_ROWS, NKT * SC], I32)
                    for kt in range(NKT):
                        nc.vector.tensor_scalar(
                            y32[:, kt * SC:(kt + 1) * SC], th3[:],
                            pc_t[:, kt:kt + 1], 0.25 * TWO16, ALU.mult, ALU.add,
                        )
                    tm = tp.tile([KT_ROWS, NKT * SC], F16)
                    yv = y32[:].bitcast(I16).rearrange("p (n two) -> p n two", two=2)[:, :, 0]
                    nc.scalar.activation(tm[:], yv, AF.Sin, scale=float(2 * np.pi / TWO16))

                    for m in range(2):
                        for half in range(2):
                            ps = pp.tile([128, 512], F32)
                            for kt in range(NKT):
                                nc.tensor.matmul(
                                    ps[:],
                                    w_t[:, kt * OUTPUT_DIM + m * 128: kt * OUTPUT_DIM + m * 128 + 128],
                                    tm[:, kt * SC + half * 512: kt * SC + half * 512 + 512],
                                    start=(kt == 0), stop=(kt == NKT - 1),
                                )
                            ob = op.tile([128, 512], F32)
                            nc.vector.tensor_copy(ob[:], ps[:])
                            nc.sync.dma_start(
                                out_d[b, m * 128:(m + 1) * 128,
                                      sc * SC + half * 512: sc * SC + half * 512 + 512],
                                ob[:],
                            )
    nc.compile()
    return nc


def _host_prep(coefficients):
    import ml_dtypes
    w = (coefficients.astype(np.float64) * WEIGHT_MAGNITUDE).astype(np.float32)
    # w: (256, 3, 257) -> lhsT rows j (i=j//42, p=42*kt+j%42), cols kt*256+o
    wk = np.zeros((KT_ROWS, NKT * OUTPUT_DIM), np.float32)
    j = np.arange(KT_ROWS)
    ii = j // 42
    for kt in range(NKT):
        pp_ = 42 * kt + (j % 42)
        valid = pp_ <= POLY_DEGREE
        # wk[j, kt*256 + o] = w[o, ii[j], pp_[j]]
        wk[valid, kt * OUTPUT_DIM:(kt + 1) * OUTPUT_DIM] = \
            w[:, ii[valid], pp_[valid]].T
    pc = np.zeros((KT_ROWS, NKT), np.float32)
    for kt in range(NKT):
        pc[:, kt] = 42 * kt + (j % 42)
    return wk.astype(ml_dtypes.float16 if False else np.float16), pc


def kernel(x, coefficients):
    from concourse.bass_utils import run_bass_kernel_spmd

    if "nc" not in _compiled:
        _compiled["nc"] = _build()
    nc = _compiled["nc"]

    wk, pc = _host_prep(coefficients)
    in_maps = []
    for c in range(N_CORES):
        xs = np.ascontiguousarray(x[:, :, c * S_SHARD:(c + 1) * S_SHARD], dtype=np.float32)
        in_maps.append({"x": xs, "w": wk, "pc": pc})
    res = run_bass_kernel_spmd(nc, in_maps, list(range(N_CORES)))
    out = np.concatenate([res.results[c]["out"] for c in range(N_CORES)], axis=2)
    return out.astype(np.float32)
